# revision 27
# baseline (speedup 1.0000x reference)
"""Trainium2 Bass kernel for the Neural-CDE-style cell (nn_JaCDE_88167088653055).

Math (per batch row b):
    x    = spline(coeffs, t)   xdot = spline(dcoeffs, t)
    l1   = x @ wx.T + h @ wh.T + b0
    relu = relu(l1);  drelu = sigmoid(l1)
    lout = relu @ wout.T + b1; th = tanh(lout); dth = 1 - th^2
    J(v) = dth * ((drelu * v) @ wout.T)        # action of the Jacobian factor
    jx   = J(xdot @ wx.T); jxh = J(jx @ wh.T); jxhh = J(jxh @ wh.T)
    out  = jx + jxh + jxhh

Device-side reformulation (all bf16 on the PE path; tolerance is 2e-2):
  * spline eval (4-term polynomial over host-gathered coeffs) runs host-side;
    x/xdot are [64, N] and stack on partitions 0:64 / 64:128 of one tile, so
    their two K=64 matmuls run in different PE row groups.
  * sign-flip trick: the xdot weight copy and the wh copy used by the
    Jacobian-chain matmuls are negated HOST-side, so every m_i arrives
    negated and dth*m_i == (th^2-1)*(-m_i) is a single STT per term, with
    th^2 from ACT Square (tanh and square share one ACT table set).
  * PAIR FUSION: batch chunks are processed in fused pairs - matmuls run at
    N=256 (PSUM-bank limit) writing adjacent halves of one [128,512] PSUM
    bank, while every elementwise op (ACT/DVE/GpSimd) covers the whole
    [128,512] pair in ONE instruction, amortizing the per-op fixed cost.
    All PSUM reads are full-range, so no reader ever touches a bank that a
    matmul is still writing (PSUM bank collisions are fatal on TRN2).
  * the two pairs are issued stage-major so each engine's strict-FIFO queue
    always holds the other pair's op to fill dependency stalls.
  * PE warm-up burst: ~3.5us of dummy matmuls during the input-DMA wait flip
    the HAM clock gate to 2.4 GHz before real work arrives (this kernel's
    matmuls are otherwise too sparse to ever leave the cold 1.2 GHz state).

Sharding: pure data parallel - batch 8192 split as 1024 rows per core across
8 cores; small weights replicated. Activations are feature-major
([feature<=128 partitions, batch free]); every matmul is out.T = W @ act.T
with the contraction on partitions.
"""

import ml_dtypes
import numpy as np

import concourse.bass as bass
import concourse.mybir as mybir
import concourse.tile as tile
from concourse import bacc, bass_utils

N_CORES = 8
B = 8192
NOBS = 16
CIN = 64
H = 128
BS = B // N_CORES       # 1024 batch rows per core
CHUNK = 256             # batch columns per matmul (one PSUM half-bank)
PAIR = 2 * CHUNK        # 512: columns per fused elementwise op
NPAIR = BS // PAIR      # 2 fused pairs per core
F32 = mybir.dt.float32
BF16 = mybir.dt.bfloat16
NPBF = ml_dtypes.bfloat16

# input pack (bf16, per pair): [128, 2*PAIR]
#   cols [0:P)    partitions 0:64 = x.T, partitions 64:128 = xdot.T
#   cols [P:2P)   h.T
PACKW = 2 * PAIR

_NC_CACHE = {}


def _build_nc():
    AF = mybir.ActivationFunctionType
    OP = mybir.AluOpType

    nc = bacc.Bacc("TRN2", target_bir_lowering=False, debug=False,
                   enable_asserts=False, num_devices=N_CORES)

    inb = nc.dram_tensor("inb", [NPAIR, 128, PACKW], BF16,
                         kind="ExternalInput")
    # [wxx2 | wh | -wh | wout] as lhsT blocks; wxx2 rows 0:64 = wx-fold for x,
    # rows 64:128 = NEGATED wx-fold for xdot.
    wpack = nc.dram_tensor("wpack", [128, 4 * H], BF16, kind="ExternalInput")
    bpack = nc.dram_tensor("bpack", [128, 2], F32, kind="ExternalInput")
    outt = nc.dram_tensor("outt", [H, BS], BF16, kind="ExternalOutput")

    def mm(out_ap, lhsT, rhs, start=True, stop=True):
        nc.tensor.matmul(out_ap, lhsT, rhs, start=start, stop=stop,
                         skip_group_check=True)

    with tile.TileContext(nc) as tc:
        with tc.tile_pool(name="w", bufs=1) as wp, \
             tc.tile_pool(name="io", bufs=2) as io, \
             tc.tile_pool(name="tmp", bufs=2) as tmp, \
             tc.tile_pool(name="ps", bufs=2, space="PSUM") as ps, \
             tc.tile_pool(name="psc", bufs=4, space="PSUM") as psc:

            # --- constants --------------------------------------------------
            ws = wp.tile([128, 4 * H], BF16, tag="ws")
            nc.scalar.dma_start(ws[:], wpack[:])
            bs_ = wp.tile([128, 2], F32, tag="bs")
            nc.scalar.dma_start(bs_[:], bpack[:])
            wxx = ws[:, 0:H]            # [128, 128]: top 64 rows x, bottom -xdot
            whs = ws[:, H:2 * H]        # +wh (for l1)
            whsn = ws[:, 2 * H:3 * H]   # -wh (for the Jacobian chain)
            wos = ws[:, 3 * H:4 * H]    # wout
            b0s = bs_[:, 0:1]
            b1s = bs_[:, 1:2]

            # dummy sigmoid: forces the ACT table-set load(s) at t=0. Its
            # input is a locally-memset tile so it does not wait on any DMA.
            dum = wp.tile([128, 2], F32, tag="dum")
            nc.vector.memset(dum[:, 0:1], 0.0)
            nc.scalar.activation(dum[:, 1:2], dum[:, 0:1], AF.Sigmoid)

            # PE warm-up (see module docstring). memset on GpSimd so the
            # burst can start as soon as the preamble ends.
            wdum = wp.tile([128, 512], BF16, tag="wdum")
            nc.gpsimd.memset(wdum[:], 0.0)
            pdum = psc.tile([H, PAIR], F32, tag="chain")
            for _ in range(8):
                mm(pdum[:], wdum[:, 0:128], wdum[:], start=True, stop=True)

            # all input DMAs issued up front on the Sync queue
            its = []
            for p in range(NPAIR):
                it = io.tile([128, PACKW], BF16, tag="it")
                nc.sync.dma_start(it[:], inb[p])
                its.append(it)

            T = {}  # (name, pair) -> tile

            def tt(name, p, dtype=BF16):
                t = tmp.tile([H, PAIR], dtype, tag=name)
                T[(name, p)] = t
                return t

            def halves(ap):
                return (ap[:, 0:CHUNK], ap[:, CHUNK:PAIR])

            def cmm(tag, p, lhsT, rhs_ab, negw=False):
                """fused chain tile: two N=256 matmuls into one bank."""
                t = psc.tile([H, PAIR], F32, tag="chain")
                T[(tag, p)] = t
                for half, rhs in zip(halves(t[:]), rhs_ab):
                    mm(half, lhsT, rhs, start=True, stop=True)
                return t

            pairs = range(NPAIR)
            for p in pairs:
                it = its[p]
                # per-chunk input slices: chunk a = cols 0:C of both blocks
                xxd_a = it[0:128, 0:CHUNK]
                xxd_b = it[0:128, CHUNK:PAIR]
                hts_a = it[:, PAIR:PAIR + CHUNK]
                hts_b = it[:, PAIR + CHUNK:2 * PAIR]

                u = ps.tile([H, PAIR], F32, tag="u")
                T[("u", p)] = u
                l1 = ps.tile([H, PAIR], F32, tag="l1")
                T[("l1", p)] = l1
                for (uh, l1h, xxd, hts) in (
                        (u[:, 0:CHUNK], l1[:, 0:CHUNK], xxd_a, hts_a),
                        (u[:, CHUNK:PAIR], l1[:, CHUNK:PAIR], xxd_b, hts_b)):
                    mm(uh, wxx[64:128, :], xxd[64:128, :], start=True,
                       stop=True)
                    mm(l1h, wxx[0:64, :], xxd[0:64, :], start=True,
                       stop=False)
                    mm(l1h, whs, hts, start=False, stop=True)

            # FRONT, pair-major: pair0's whole ACT sequence (relu->dr->th->sq,
            # the longest pole gating its Jacobian chain) must sit ahead of
            # pair1's in the Scalar FIFO, and sq(p0) gates jx(p0). The
            # scheduler's readiness heap reorders this unless pair0's front
    	    # gets explicit priority.
            for p in pairs:
                import contextlib
                prio = tc.high_priority(offset=30) if p == 0 \
                    else contextlib.nullcontext()
                with prio:
                    nc.scalar.activation(tt("relu", p)[:], T[("l1", p)][:],
                                         AF.Relu, bias=b0s)
                    nc.scalar.activation(tt("dr", p)[:], T[("l1", p)][:],
                                         AF.Sigmoid, bias=b0s)
                    cmm("lout", p, wos, halves(T[("relu", p)][:]))
                    nc.scalar.activation(tt("th", p)[:], T[("lout", p)][:],
                                         AF.Tanh, bias=b1s)
                    nc.scalar.activation(tt("sq", p)[:], T[("th", p)][:],
                                         AF.Square)
                    nc.vector.tensor_mul(tt("p1", p)[:], T[("dr", p)][:],
                                         T[("u", p)][:])
                    cmm("m1", p, wos, halves(T[("p1", p)][:]))  # = -m1
            for p in pairs:
                # jx = dth*m1 = (th^2-1)*(-m1)
                nc.vector.scalar_tensor_tensor(
                    tt("jx", p)[:], T[("sq", p)][:], 1.0,
                    T[("m1", p)][:], OP.subtract, OP.mult)
            for p in pairs:
                cmm("g1", p, whsn, halves(T[("jx", p)][:]))     # = -g1
            for p in pairs:
                nc.vector.tensor_mul(tt("p2", p)[:], T[("dr", p)][:],
                                     T[("g1", p)][:])           # = -p2
            for p in pairs:
                cmm("m2", p, wos, halves(T[("p2", p)][:]))      # = -m2
            for p in pairs:
                nc.vector.scalar_tensor_tensor(
                    tt("jxh", p)[:], T[("sq", p)][:], 1.0,
                    T[("m2", p)][:], OP.subtract, OP.mult)
            for p in pairs:
                nc.gpsimd.tensor_add(tt("t12", p)[:], T[("jx", p)][:],
                                     T[("jxh", p)][:])
            for p in pairs:
                cmm("g2", p, whsn, halves(T[("jxh", p)][:]))    # = -g2
            for p in pairs:
                nc.vector.tensor_mul(tt("p3", p)[:], T[("dr", p)][:],
                                     T[("g2", p)][:])           # = -p3
            for p in pairs:
                cmm("m3", p, wos, halves(T[("p3", p)][:]))      # = -m3
            for p in pairs:
                last = p == NPAIR - 1
                outs = tt("outs", p)
                if not last:
                    nc.vector.scalar_tensor_tensor(
                        tt("jxhh", p)[:], T[("sq", p)][:], 1.0,
                        T[("m3", p)][:], OP.subtract, OP.mult)
                    nc.gpsimd.tensor_add(outs[:], T[("t12", p)][:],
                                         T[("jxhh", p)][:])
                    nc.sync.dma_start(outt[:, bass.ts(p, PAIR)], outs[:])
                else:
                    # the last pair gates the kernel tail: after the (full
                    # bank-range - half-range PSUM reads race the twin
                    # matmul's writes) STT, do the final SBUF-only sum in
                    # [H,256] halves on the DVE (bf16 SBUF adds get the 2x
                    # mode at 256) and stream each half's output DMA as soon
                    # as it is ready.
                    jxhh = tt("jxhh", p)
                    nc.vector.scalar_tensor_tensor(
                        jxhh[:], T[("sq", p)][:], 1.0,
                        T[("m3", p)][:], OP.subtract, OP.mult)
                    for hi in range(2):
                        hs = slice(hi * CHUNK, (hi + 1) * CHUNK)
                        nc.vector.tensor_add(outs[:, hs],
                                             T[("t12", p)][:, hs],
                                             jxhh[:, hs])
                        nc.sync.dma_start(
                            outt[:, bass.ts(2 * p + hi, CHUNK)], outs[:, hs])

    nc.compile()
    return nc


def _get_nc():
    if "nc" not in _NC_CACHE:
        _NC_CACHE["nc"] = _build_nc()
    return _NC_CACHE["nc"]


def _prep_in_maps(t, h, coeffs, dcoeffs, tobs, wx, wh, wout, b0, b1):
    t = np.asarray(t, np.float32)
    h = np.asarray(h, np.float32)
    coeffs = np.asarray(coeffs, np.float32)
    dcoeffs = np.asarray(dcoeffs, np.float32)
    tobs = np.asarray(tobs, np.float32)
    wx = np.asarray(wx, np.float32)
    wh = np.asarray(wh, np.float32)
    wout = np.asarray(wout, np.float32)
    b0 = np.asarray(b0, np.float32)
    b1 = np.asarray(b1, np.float32)

    ts = t[0]
    idx = int(np.clip(np.searchsorted(tobs, ts, side="right") - 1, 0, NOBS - 2))
    dtv = np.float32(ts - tobs[idx])
    powers = dtv ** np.arange(4, dtype=np.float32)            # [4]

    # host-side spline eval: x[b,c] = sum_j coeffs[b,idx,c,j] * dt^j
    x = coeffs[:, idx] @ powers                               # [B, CIN]
    xdot = dcoeffs[:, idx] @ powers                           # [B, CIN]

    # weights pack [128, 512] bf16: [wxx2 | wh.T | -wh.T | wout.T]
    wxx2 = np.concatenate([wx.T, -wx.T], axis=0)              # [128, 128]
    wpack = np.concatenate([wxx2, wh.T, -wh.T, wout.T],
                           axis=1).astype(NPBF)
    bpack = np.stack([b0, b1], axis=1).astype(np.float32)     # [128, 2]

    xb = x.astype(NPBF)
    xdb = xdot.astype(NPBF)
    hb = h.astype(NPBF)

    in_maps = []
    for c in range(N_CORES):
        sl = slice(c * BS, (c + 1) * BS)
        xt = xb[sl].T                                         # [64, BS]
        xdt = xdb[sl].T
        ht = hb[sl].T                                         # [128, BS]
        inb = np.empty((NPAIR, 128, PACKW), NPBF)
        for p in range(NPAIR):
            cls = slice(p * PAIR, (p + 1) * PAIR)
            inb[p, 0:64, 0:PAIR] = xt[:, cls]
            inb[p, 64:128, 0:PAIR] = xdt[:, cls]
            inb[p, :, PAIR:2 * PAIR] = ht[:, cls]
        in_maps.append({"inb": inb, "wpack": wpack, "bpack": bpack})
    return in_maps


def kernel(**inputs) -> np.ndarray:
    in_maps = _prep_in_maps(**inputs)
    nc = _get_nc()
    res = bass_utils.run_bass_kernel_spmd(nc, in_maps,
                                          core_ids=list(range(N_CORES)))
    out = np.empty((B, H), np.float32)
    for c in range(N_CORES):
        out[c * BS:(c + 1) * BS] = res.results[c]["outt"].T.astype(np.float32)
    return out


# revision 30
# speedup vs baseline: 1.0213x; 1.0213x over previous
"""Trainium2 Bass kernel for the Neural-CDE-style cell (nn_JaCDE_88167088653055).

Math (per batch row b):
    x    = spline(coeffs, t)   xdot = spline(dcoeffs, t)
    l1   = x @ wx.T + h @ wh.T + b0
    relu = relu(l1);  drelu = sigmoid(l1)
    lout = relu @ wout.T + b1; th = tanh(lout); dth = 1 - th^2
    J(v) = dth * ((drelu * v) @ wout.T)        # action of the Jacobian factor
    jx   = J(xdot @ wx.T); jxh = J(jx @ wh.T); jxhh = J(jxh @ wh.T)
    out  = jx + jxh + jxhh

Device-side reformulation (all bf16 on the PE path; tolerance is 2e-2):
  * spline eval (4-term polynomial over host-gathered coeffs) runs host-side;
    x/xdot are [64, N] and stack on partitions 0:64 / 64:128 of one tile, so
    their two K=64 matmuls run in different PE row groups.
  * sign-flip trick: the xdot weight copy and the wh copy used by the
    Jacobian-chain matmuls are negated HOST-side, so every m_i arrives
    negated and dth*m_i == (th^2-1)*(-m_i) is a single STT per term, with
    th^2 from ACT Square (tanh and square share one ACT table set).
  * PAIR FUSION: batch chunks are processed in fused pairs - matmuls run at
    N=256 (PSUM-bank limit) writing adjacent halves of one [128,512] PSUM
    bank, while every elementwise op (ACT/DVE/GpSimd) covers the whole
    [128,512] pair in ONE instruction, amortizing the per-op fixed cost.
    All PSUM reads are full-range, so no reader ever touches a bank that a
    matmul is still writing (PSUM bank collisions are fatal on TRN2).
  * the two pairs are issued stage-major so each engine's strict-FIFO queue
    always holds the other pair's op to fill dependency stalls.
  * PE warm-up burst: ~3.5us of dummy matmuls during the input-DMA wait flip
    the HAM clock gate to 2.4 GHz before real work arrives (this kernel's
    matmuls are otherwise too sparse to ever leave the cold 1.2 GHz state).

Sharding: pure data parallel - batch 8192 split as 1024 rows per core across
8 cores; small weights replicated. Activations are feature-major
([feature<=128 partitions, batch free]); every matmul is out.T = W @ act.T
with the contraction on partitions.
"""

import ml_dtypes
import numpy as np

import concourse.bass as bass
import concourse.mybir as mybir
import concourse.tile as tile
from concourse import bacc, bass_utils

N_CORES = 8
B = 8192
NOBS = 16
CIN = 64
H = 128
BS = B // N_CORES       # 1024 batch rows per core
CHUNK = 256             # batch columns per matmul (one PSUM half-bank)
PAIR = 2 * CHUNK        # 512: columns per fused elementwise op
NPAIR = BS // PAIR      # 2 fused pairs per core
F32 = mybir.dt.float32
BF16 = mybir.dt.bfloat16
NPBF = ml_dtypes.bfloat16

# input pack (bf16, per pair): [128, 2*PAIR]
#   cols [0:P)    partitions 0:64 = x.T, partitions 64:128 = xdot.T
#   cols [P:2P)   h.T
PACKW = 2 * PAIR

_NC_CACHE = {}


def _build_nc():
    AF = mybir.ActivationFunctionType
    OP = mybir.AluOpType

    nc = bacc.Bacc("TRN2", target_bir_lowering=False, debug=False,
                   enable_asserts=False, num_devices=N_CORES)

    inb = nc.dram_tensor("inb", [NPAIR, 128, PACKW], BF16,
                         kind="ExternalInput")
    # [wxx2 | wh | -wh | wout] as lhsT blocks; wxx2 rows 0:64 = wx-fold for x,
    # rows 64:128 = NEGATED wx-fold for xdot.
    wpack = nc.dram_tensor("wpack", [128, 4 * H], BF16, kind="ExternalInput")
    bpack = nc.dram_tensor("bpack", [128, 2], F32, kind="ExternalInput")
    outt = nc.dram_tensor("outt", [H, BS], BF16, kind="ExternalOutput")

    def mm(out_ap, lhsT, rhs, start=True, stop=True):
        nc.tensor.matmul(out_ap, lhsT, rhs, start=start, stop=stop,
                         skip_group_check=True)

    with tile.TileContext(nc) as tc:
        with tc.tile_pool(name="w", bufs=1) as wp, \
             tc.tile_pool(name="io", bufs=2) as io, \
             tc.tile_pool(name="tmp", bufs=2) as tmp, \
             tc.tile_pool(name="ps", bufs=2, space="PSUM") as ps, \
             tc.tile_pool(name="psc", bufs=4, space="PSUM") as psc:

            # --- constants --------------------------------------------------
            ws = wp.tile([128, 4 * H], BF16, tag="ws")
            nc.scalar.dma_start(ws[:], wpack[:])
            bs_ = wp.tile([128, 2], F32, tag="bs")
            nc.scalar.dma_start(bs_[:], bpack[:])
            wxx = ws[:, 0:H]            # [128, 128]: top 64 rows x, bottom -xdot
            whs = ws[:, H:2 * H]        # +wh (for l1)
            whsn = ws[:, 2 * H:3 * H]   # -wh (for the Jacobian chain)
            wos = ws[:, 3 * H:4 * H]    # wout
            b0s = bs_[:, 0:1]
            b1s = bs_[:, 1:2]

            # dummy sigmoid: forces the ACT table-set load(s) at t=0.
            dum = wp.tile([128, 1], F32, tag="dum")
            nc.scalar.activation(dum[:], bs_[:, 0:1], AF.Sigmoid)

            # PE warm-up (see module docstring). memset on GpSimd so the
            # burst can start as soon as the preamble ends.
            wdum = wp.tile([128, 512], BF16, tag="wdum")
            nc.gpsimd.memset(wdum[:], 0.0)
            pdum = psc.tile([H, PAIR], F32, tag="chain")
            for _ in range(8):
                mm(pdum[:], wdum[:, 0:128], wdum[:], start=True, stop=True)

            # all input DMAs issued up front on the Sync queue
            its = []
            for p in range(NPAIR):
                it = io.tile([128, PACKW], BF16, tag="it")
                nc.sync.dma_start(it[:], inb[p])
                its.append(it)

            T = {}  # (name, pair) -> tile

            def tt(name, p, dtype=BF16):
                t = tmp.tile([H, PAIR], dtype, tag=name)
                T[(name, p)] = t
                return t

            def halves(ap):
                return (ap[:, 0:CHUNK], ap[:, CHUNK:PAIR])

            def cmm(tag, p, lhsT, rhs_ab, negw=False):
                """fused chain tile: two N=256 matmuls into one bank."""
                t = psc.tile([H, PAIR], F32, tag="chain")
                T[(tag, p)] = t
                for half, rhs in zip(halves(t[:]), rhs_ab):
                    mm(half, lhsT, rhs, start=True, stop=True)
                return t

            pairs = range(NPAIR)
            for p in pairs:
                it = its[p]
                # per-chunk input slices: chunk a = cols 0:C of both blocks
                xxd_a = it[0:128, 0:CHUNK]
                xxd_b = it[0:128, CHUNK:PAIR]
                hts_a = it[:, PAIR:PAIR + CHUNK]
                hts_b = it[:, PAIR + CHUNK:2 * PAIR]

                u = ps.tile([H, PAIR], F32, tag="u")
                T[("u", p)] = u
                l1 = ps.tile([H, PAIR], F32, tag="l1")
                T[("l1", p)] = l1
                for (uh, l1h, xxd, hts) in (
                        (u[:, 0:CHUNK], l1[:, 0:CHUNK], xxd_a, hts_a),
                        (u[:, CHUNK:PAIR], l1[:, CHUNK:PAIR], xxd_b, hts_b)):
                    mm(uh, wxx[64:128, :], xxd[64:128, :], start=True,
                       stop=True)
                    mm(l1h, wxx[0:64, :], xxd[0:64, :], start=True,
                       stop=False)
                    mm(l1h, whs, hts, start=False, stop=True)

            # FRONT, pair-major: pair0's whole ACT sequence (relu->dr->th->sq,
            # the longest pole gating its Jacobian chain) should sit ahead of
            # pair1's in the Scalar FIFO, and sq(p0) gates jx(p0).
            for p in pairs:
                nc.scalar.activation(tt("relu", p)[:], T[("l1", p)][:],
                                     AF.Relu, bias=b0s)
                nc.scalar.activation(tt("dr", p)[:], T[("l1", p)][:],
                                     AF.Sigmoid, bias=b0s)
                cmm("lout", p, wos, halves(T[("relu", p)][:]))
                nc.scalar.activation(tt("th", p)[:], T[("lout", p)][:],
                                     AF.Tanh, bias=b1s)
                nc.scalar.activation(tt("sq", p)[:], T[("th", p)][:],
                                     AF.Square)
                nc.vector.tensor_mul(tt("p1", p)[:], T[("dr", p)][:],
                                     T[("u", p)][:])
                cmm("m1", p, wos, halves(T[("p1", p)][:]))      # = -m1
            for p in pairs:
                # jx = dth*m1 = (th^2-1)*(-m1)
                nc.vector.scalar_tensor_tensor(
                    tt("jx", p)[:], T[("sq", p)][:], 1.0,
                    T[("m1", p)][:], OP.subtract, OP.mult)
            for p in pairs:
                cmm("g1", p, whsn, halves(T[("jx", p)][:]))     # = -g1
            for p in pairs:
                nc.vector.tensor_mul(tt("p2", p)[:], T[("dr", p)][:],
                                     T[("g1", p)][:])           # = -p2
            for p in pairs:
                cmm("m2", p, wos, halves(T[("p2", p)][:]))      # = -m2
            for p in pairs:
                nc.vector.scalar_tensor_tensor(
                    tt("jxh", p)[:], T[("sq", p)][:], 1.0,
                    T[("m2", p)][:], OP.subtract, OP.mult)
            for p in pairs:
                nc.gpsimd.tensor_add(tt("t12", p)[:], T[("jx", p)][:],
                                     T[("jxh", p)][:])
            for p in pairs:
                cmm("g2", p, whsn, halves(T[("jxh", p)][:]))    # = -g2
            for p in pairs:
                nc.vector.tensor_mul(tt("p3", p)[:], T[("dr", p)][:],
                                     T[("g2", p)][:])           # = -p3
            for p in pairs:
                cmm("m3", p, wos, halves(T[("p3", p)][:]))      # = -m3
            for p in pairs:
                nc.vector.scalar_tensor_tensor(
                    tt("jxhh", p)[:], T[("sq", p)][:], 1.0,
                    T[("m3", p)][:], OP.subtract, OP.mult)
            for p in pairs:
                outs = tt("outs", p)
                if p == NPAIR - 1:
                    # last pair's sum gates the kernel tail: DVE is faster
                    # than GpSimd for this bf16 SBUF add.
                    nc.vector.tensor_add(outs[:], T[("t12", p)][:],
                                         T[("jxhh", p)][:])
                else:
                    nc.gpsimd.tensor_add(outs[:], T[("t12", p)][:],
                                         T[("jxhh", p)][:])
            for p in pairs:
                nc.sync.dma_start(outt[:, bass.ts(p, PAIR)],
                                  T[("outs", p)][:])

    nc.compile()
    return nc


def _get_nc():
    if "nc" not in _NC_CACHE:
        _NC_CACHE["nc"] = _build_nc()
    return _NC_CACHE["nc"]


def _prep_in_maps(t, h, coeffs, dcoeffs, tobs, wx, wh, wout, b0, b1):
    t = np.asarray(t, np.float32)
    h = np.asarray(h, np.float32)
    coeffs = np.asarray(coeffs, np.float32)
    dcoeffs = np.asarray(dcoeffs, np.float32)
    tobs = np.asarray(tobs, np.float32)
    wx = np.asarray(wx, np.float32)
    wh = np.asarray(wh, np.float32)
    wout = np.asarray(wout, np.float32)
    b0 = np.asarray(b0, np.float32)
    b1 = np.asarray(b1, np.float32)

    ts = t[0]
    idx = int(np.clip(np.searchsorted(tobs, ts, side="right") - 1, 0, NOBS - 2))
    dtv = np.float32(ts - tobs[idx])
    powers = dtv ** np.arange(4, dtype=np.float32)            # [4]

    # host-side spline eval: x[b,c] = sum_j coeffs[b,idx,c,j] * dt^j
    x = coeffs[:, idx] @ powers                               # [B, CIN]
    xdot = dcoeffs[:, idx] @ powers                           # [B, CIN]

    # weights pack [128, 512] bf16: [wxx2 | wh.T | -wh.T | wout.T]
    wxx2 = np.concatenate([wx.T, -wx.T], axis=0)              # [128, 128]
    wpack = np.concatenate([wxx2, wh.T, -wh.T, wout.T],
                           axis=1).astype(NPBF)
    bpack = np.stack([b0, b1], axis=1).astype(np.float32)     # [128, 2]

    xb = x.astype(NPBF)
    xdb = xdot.astype(NPBF)
    hb = h.astype(NPBF)

    in_maps = []
    for c in range(N_CORES):
        sl = slice(c * BS, (c + 1) * BS)
        xt = xb[sl].T                                         # [64, BS]
        xdt = xdb[sl].T
        ht = hb[sl].T                                         # [128, BS]
        inb = np.empty((NPAIR, 128, PACKW), NPBF)
        for p in range(NPAIR):
            cls = slice(p * PAIR, (p + 1) * PAIR)
            inb[p, 0:64, 0:PAIR] = xt[:, cls]
            inb[p, 64:128, 0:PAIR] = xdt[:, cls]
            inb[p, :, PAIR:2 * PAIR] = ht[:, cls]
        in_maps.append({"inb": inb, "wpack": wpack, "bpack": bpack})
    return in_maps


def kernel(**inputs) -> np.ndarray:
    in_maps = _prep_in_maps(**inputs)
    nc = _get_nc()
    res = bass_utils.run_bass_kernel_spmd(nc, in_maps,
                                          core_ids=list(range(N_CORES)))
    out = np.empty((B, H), np.float32)
    for c in range(N_CORES):
        out[c * BS:(c + 1) * BS] = res.results[c]["outt"].T.astype(np.float32)
    return out
